# revision 4
# baseline (speedup 1.0000x reference)
"""TRN2 Bass kernel for nn_BottleneckA — fp8 DoubleRow version.

Computation (reference):
    h1 = relu(g * (W1 @ x))    g = relu(gate)   per (batch, mid-channel)
    h2 = relu(g * (W2 @ h1))
    y  = relu(W3 @ h2) + x     (all biases are zero in this problem)

Key restructuring vs the bf16/f32r baseline:
  * relu(g*z) = g*relu(z) for g>=0, so the per-(batch,channel) gate folds
    into the columns of W2 and W3 (per-batch weight copies, tiny at 1x1):
        r1 = relu(W1 x);  r2 = relu((W2*g) r1);  y = relu((W3*g) r2) + x
    leaving every non-matmul pass a plain relu with a scalar immediate scale.
  * All three convs run as fp8e4m3 DoubleRow matmuls (K=256 contracted per
    instruction at >= 2x bf16 PE rate). Power-of-two tensor scales keep
    e4m3 quantization in range; measured end-to-end rel-err ~8e-3.
  * x ships to the device as fp8 (half the bf16 bytes); the device returns
    relu3 in fp8; the exact fp32 residual `+ x` is applied on the host.
  * Elementwise drains split across ACT (r1/r2) and DVE/GPSIMD (conv3 out)
    so no single engine backlogs the PE.

Sharding: data-parallel over batch B=16 across 8 NeuronCores (2 per core),
each batch's [1024, 3136] activation processed in 7 chunks of 448 columns.
"""
import os
import time

import numpy as np

import concourse.bass as bass
import concourse.tile as tile
from concourse import mybir, bass2jax
from concourse.bass2jax import _bass_exec_p, install_neuronx_cc_hook
from contextlib import ExitStack

import jax
from jax.sharding import Mesh, PartitionSpec
from jax.experimental.shard_map import shard_map

B, C, MID, HW = 16, 1024, 256, 56 * 56
NCORES = 8
BPC = B // NCORES            # batches per core
NC_CHUNK = 448               # spatial chunk width (3136 = 7 * 448)
NCHUNKS = HW // NC_CHUNK
KO = C // 128                # 8 input k-tiles
J1 = KO // 2                 # 4 DoubleRow k-pairs for conv1
M2 = MID // 128              # 2 mid m-tiles
f32 = mybir.dt.float32
fp8 = mybir.dt.float8e4

# Power-of-two scales: value*S stored in e4m3. Folded into the matmul
# operands on the host and undone by the relu-pass scalar immediates.
SX, SW1, SR1 = 16.0, 256.0, 16.0
SW2, SR2 = 256.0, 32.0
SW3, SY = 256.0, 32.0
S1 = SR1 / (SX * SW1)        # 1/256
S2 = SR2 / (SR1 * SW2)       # 1/128
S3 = SY / (SR2 * SW3)        # 1/256

# conv3 drain engine per m-tile: v=DVE, a=ACT (GPSIMD cannot access PSUM)
DRAIN_PLAN = os.environ.get("BOTTLENECK_DRAIN", "vvavvavv")

_EVS_CAP = 2


def _split_excess_waits(nc):
    """This container's walrus accepts only 1 sync-wait slot on most ISA
    structs while Tile emits 2-3; hoist the excess onto preceding
    InstEventSemaphore ops on the same (FIFO) engine queue."""
    counter = [0]
    for fn in nc.m.functions:
        for blk in fn.blocks:
            new_insts = []
            for inst in blk.instructions:
                si = inst.sync_info
                waits = list(si.on_wait) if si is not None else []
                cap = _EVS_CAP if type(inst).__name__ == "InstEventSemaphore" else 1
                if len(waits) > cap:
                    excess, keep = waits[: len(waits) - cap], waits[len(waits) - cap:]
                    while excess:
                        chunk, excess = excess[:_EVS_CAP], excess[_EVS_CAP:]
                        counter[0] += 1
                        new_insts.append(mybir.InstEventSemaphore(
                            name=f"EVSW-{counter[0]}-{inst.name}",
                            engine=inst.engine,
                            ins=[], outs=[],
                            sync_info=mybir.SyncInfo(on_wait=list(chunk), on_update=[]),
                        ))
                    inst.sync_info = mybir.SyncInfo(
                        on_wait=keep, on_update=list(si.on_update))
                new_insts.append(inst)
            blk.instructions = new_insts


def build_bass(repeat: int = 1) -> bass.Bass:
    nc = bass.Bass(trn_type="TRN2")
    xs = nc.dram_tensor("xs", [BPC, C, HW], fp8, kind="ExternalInput")
    w1t = nc.dram_tensor("w1t", [J1, 2, M2, 128, 128], fp8, kind="ExternalInput")
    w2g = nc.dram_tensor("w2g", [BPC, 2, M2, 128, 128], fp8, kind="ExternalInput")
    w3g = nc.dram_tensor("w3g", [BPC, 2, KO, 128, 128], fp8, kind="ExternalInput")
    # Device returns relu(W3g r2)*S3-ish in fp8; host applies + x in fp32.
    ys = nc.dram_tensor("ys", [BPC, C, HW], fp8, kind="ExternalOutput")

    Relu = mybir.ActivationFunctionType.Relu
    DR = mybir.MatmulPerfMode.DoubleRow

    with tile.TileContext(nc) as tc, ExitStack() as ctx:
        wpool = ctx.enter_context(tc.tile_pool(name="w", bufs=1))
        xpool = ctx.enter_context(tc.tile_pool(name="x", bufs=6))
        rpool = ctx.enter_context(tc.tile_pool(name="r", bufs=4))
        opool = ctx.enter_context(tc.tile_pool(name="o", bufs=3))
        pp1 = ctx.enter_context(tc.tile_pool(name="pp1", bufs=3, space="PSUM"))
        pp2 = ctx.enter_context(tc.tile_pool(name="pp2", bufs=2, space="PSUM"))
        pp3 = ctx.enter_context(tc.tile_pool(name="pp3", bufs=3, space="PSUM"))

        # x loads ride the SP HWDGE ring; weight loads + output stores ride
        # the ACT ring so the two streams don't queue behind each other.
        w1_sb = wpool.tile([128, J1, 2, M2, 128], fp8, tag="w1")
        nc.scalar.dma_start(w1_sb[:], w1t[:].rearrange("j i m p c -> p j i m c"))
        w2_sb = wpool.tile([128, BPC, 2, M2, 128], fp8, tag="w2")
        nc.scalar.dma_start(w2_sb[:], w2g[:].rearrange("b i m p c -> p b i m c"))
        w3_sb = wpool.tile([128, BPC, 2, KO, 128], fp8, tag="w3")
        nc.scalar.dma_start(w3_sb[:], w3g[:].rearrange("b i m p c -> p b i m c"))

        chunks = [(b, ci * NC_CHUNK) for b in range(BPC) for ci in range(NCHUNKS)]

        def emit_load(i, halves=1):
            b, n0 = chunks[i]
            x_t = xpool.tile([128, KO, NC_CHUNK], fp8, tag="xt")
            src = xs[b][:, n0:n0 + NC_CHUNK].rearrange("(ko p) n -> p ko n", p=128)
            step = KO // halves
            for h in range(halves):
                nc.sync.dma_start(x_t[:, h * step:(h + 1) * step, :],
                                  src[:, h * step:(h + 1) * step, :])
            return x_t

        def emit_conv1(x_t):
            ps1 = []
            for m in range(M2):
                ps = pp1.tile([128, NC_CHUNK], f32, tag="ps1")
                for j in range(J1):
                    nc.tensor.matmul(ps[:], w1_sb[:, j, :, m, :],
                                     x_t[:, 2 * j:2 * j + 2, :],
                                     start=(j == 0), stop=(j == J1 - 1),
                                     perf_mode=DR)
                ps1.append(ps)
            return ps1

        def emit_fin_a(i, ps1):
            """r1 = relu(S1*ps1) in fp8; conv2 (DoubleRow); r2 = relu(S2*ps2)."""
            b, _ = chunks[i]
            r1 = rpool.tile([128, 2, NC_CHUNK], fp8, tag="r1")
            for m in range(M2):
                nc.scalar.activation(r1[:, m, :], ps1[m][:], Relu, scale=S1)
            r2 = rpool.tile([128, 2, NC_CHUNK], fp8, tag="r2")
            for m in range(M2):
                ps = pp2.tile([128, NC_CHUNK], f32, tag="ps2")
                nc.tensor.matmul(ps[:], w2_sb[:, b, :, m, :], r1[:, :, :],
                                 start=True, stop=True, perf_mode=DR)
                nc.scalar.activation(r2[:, m, :], ps[:], Relu, scale=S2)
            return r2

        def emit_fin_b(i, r2, last):
            """conv3 (DoubleRow); relu(S3*ps3) in fp8 on DVE/GPSIMD; store."""
            b, n0 = chunks[i]
            o_t = opool.tile([128, KO, NC_CHUNK], fp8, tag="ot")
            dst = ys[b][:, n0:n0 + NC_CHUNK].rearrange("(m p) n -> p m n", p=128)
            half = KO // 2
            for m8 in range(KO):
                ps = pp3.tile([128, NC_CHUNK], f32, tag="ps3")
                nc.tensor.matmul(ps[:], w3_sb[:, b, :, m8, :], r2[:, :, :],
                                 start=True, stop=True, perf_mode=DR)
                if DRAIN_PLAN[m8] == "v":
                    nc.vector.tensor_scalar(o_t[:, m8, :], ps[:], S3, 0.0,
                                            mybir.AluOpType.mult,
                                            mybir.AluOpType.max)
                else:
                    nc.scalar.activation(o_t[:, m8, :], ps[:], Relu, scale=S3)
                if last and m8 == half - 1:
                    nc.scalar.dma_start(dst[:, :half, :], o_t[:, :half, :])
            if last:
                nc.scalar.dma_start(dst[:, half:, :], o_t[:, half:, :])

        n = len(chunks)
        for r in range(repeat):
            last = r == repeat - 1
            xts = {}
            ps1s = {}
            r2s = {}
            for j in range(min(4, n)):
                xts[j] = emit_load(j, halves=2 if j == 0 else 1)
            ps1s[0] = emit_conv1(xts[0])
            r2s[0] = emit_fin_a(0, ps1s.pop(0))
            if n > 1:
                ps1s[1] = emit_conv1(xts[1])
            for i in range(n):
                if i + 4 < n:
                    xts[i + 4] = emit_load(i + 4)
                if i + 1 < n:
                    r2s[i + 1] = emit_fin_a(i + 1, ps1s.pop(i + 1))
                if i + 2 < n:
                    ps1s[i + 2] = emit_conv1(xts[i + 2])
                xts.pop(i)
                emit_fin_b(i, r2s.pop(i), last)
    return nc


class _Exec:
    """Compile-once PJRT executor for the SPMD bass program (axon backend)."""

    def __init__(self, nc, n_cores):
        install_neuronx_cc_hook()
        self.n_cores = n_cores
        partition_name = nc.partition_id_tensor.name if nc.partition_id_tensor else None
        in_names, out_names, out_avals, zero_outs = [], [], [], []
        for alloc in nc.m.functions[0].allocations:
            if not isinstance(alloc, mybir.MemoryLocationSet):
                continue
            name = alloc.memorylocations[0].name
            if alloc.kind == "ExternalInput":
                if name != partition_name:
                    in_names.append(name)
            elif alloc.kind == "ExternalOutput":
                shape = tuple(alloc.tensor_shape)
                dtype = mybir.dt.np(alloc.dtype)
                out_names.append(name)
                out_avals.append(jax.core.ShapedArray(shape, dtype))
                zero_outs.append(np.zeros(shape, dtype))
        self.in_names, self.out_names, self.zero_outs = in_names, out_names, zero_outs
        n_params = len(in_names)
        all_in = list(in_names) + list(out_names)
        if partition_name is not None:
            all_in.append(partition_name)

        def _body(*args):
            operands = list(args)
            if partition_name is not None:
                operands.append(bass2jax.partition_id_tensor())
            return tuple(_bass_exec_p.bind(
                *operands,
                out_avals=tuple(out_avals),
                in_names=tuple(all_in),
                out_names=tuple(out_names),
                lowering_input_output_aliases=(),
                sim_require_finite=True,
                sim_require_nnan=True,
                nc=nc,
            ))

        devices = jax.devices()[:n_cores]
        assert len(devices) == n_cores, f"need {n_cores} cores, have {len(jax.devices())}"
        mesh = Mesh(np.asarray(devices), ("core",))
        specs = (PartitionSpec("core"),) * (n_params + len(out_names))
        self._fn = jax.jit(
            shard_map(_body, mesh=mesh, in_specs=specs,
                      out_specs=(PartitionSpec("core"),) * len(out_names),
                      check_rep=False),
            keep_unused=True,
        )

    def stage(self, in_maps):
        per_core = [[np.asarray(m[n]) for n in self.in_names] for m in in_maps]
        args = [np.concatenate([per_core[c][i] for c in range(self.n_cores)], axis=0)
                for i in range(len(self.in_names))]
        args += [np.zeros((self.n_cores * z.shape[0], *z.shape[1:]), z.dtype)
                 for z in self.zero_outs]
        return args

    def run_staged(self, args):
        out = self._fn(*args)
        jax.block_until_ready(out)
        return out

    def fetch(self, out_arrs):
        return [
            {n: np.asarray(out_arrs[i]).reshape(self.n_cores, *self.zero_outs[i].shape)[c]
             for i, n in enumerate(self.out_names)}
            for c in range(self.n_cores)
        ]


_EXEC_CACHE = {}


def _get_exec(repeat: int = 1):
    if repeat not in _EXEC_CACHE:
        nc = build_bass(repeat)
        _split_excess_waits(nc)
        _EXEC_CACHE[repeat] = _Exec(nc, NCORES)
    return _EXEC_CACHE[repeat]


def _prepare_in_maps(x, gate_values, W1, b1, W2, b2, W3, b3):
    import ml_dtypes
    e4m3 = ml_dtypes.float8_e4m3
    x = np.asarray(x, dtype=np.float32)
    gate = np.asarray(gate_values, dtype=np.float32)
    W1 = np.asarray(W1, dtype=np.float32)
    W2 = np.asarray(W2, dtype=np.float32)
    W3 = np.asarray(W3, dtype=np.float32)
    # Biases are structurally zero in this problem; the device program
    # assumes so (pure relu passes with immediate scales).
    for bv in (b1, b2, b3):
        assert not np.any(np.asarray(bv)), "nonzero bias unsupported"

    xs_all = (x.reshape(B, C, HW) * SX).astype(e4m3)
    g_all = np.maximum(gate, 0.0)                      # [B, MID]

    # lhsT DoubleRow tiles: [j, i, m, p, c] = Wq.T[(2j+i)*128+p, m*128+c]
    w1t = np.ascontiguousarray(
        (W1 * SW1).astype(e4m3).T.reshape(J1, 2, 128, M2, 128)
        .transpose(0, 1, 3, 2, 4))

    in_maps = []
    for c in range(NCORES):
        w2l, w3l = [], []
        for bl in range(BPC):
            g = g_all[c * BPC + bl]
            w2q = (W2 * g[None, :] * SW2).astype(e4m3)   # [MID, MID]
            w3q = (W3 * g[None, :] * SW3).astype(e4m3)   # [C, MID]
            w2l.append(w2q.T.reshape(2, 128, M2, 128).transpose(0, 2, 1, 3))
            w3l.append(w3q.T.reshape(2, 128, KO, 128).transpose(0, 2, 1, 3))
        in_maps.append({
            "xs": xs_all[c * BPC:(c + 1) * BPC],
            "w1t": w1t,
            "w2g": np.ascontiguousarray(np.stack(w2l)),
            "w3g": np.ascontiguousarray(np.stack(w3l)),
        })
    return in_maps


def kernel(x, gate_values, W1, b1, W2, b2, W3, b3):
    in_maps = _prepare_in_maps(x, gate_values, W1, b1, W2, b2, W3, b3)
    ex = _get_exec(int(os.environ.get("BOTTLENECK_REPEAT", "1")))
    args = ex.stage(in_maps)
    try:
        out_arrs = ex.run_staged(args)
    except Exception:
        time.sleep(2.0)  # transient device wedge: retry once
        out_arrs = ex.run_staged(args)
    outs = ex.fetch(out_arrs)
    relu3 = np.concatenate([o["ys"] for o in outs], axis=0).astype(np.float32)
    y = np.asarray(x, dtype=np.float32).reshape(B, C, HW) + relu3 * (1.0 / SY)
    return y.reshape(B, C, 56, 56)


# revision 5
# speedup vs baseline: 1.0791x; 1.0791x over previous
"""TRN2 Bass kernel for nn_BottleneckA — fp8 DoubleRow, DMA-efficient version.

Computation (reference):
    h1 = relu(g * (W1 @ x))    g = relu(gate)   per (batch, mid-channel)
    h2 = relu(g * (W2 @ h1))
    y  = relu(W3 @ h2) + x     (all biases are zero in this problem)

Design (evolved from the bf16/f32r baseline via sim-trace analysis):
  * relu(g*z) = g*relu(z) for g>=0: the per-(batch,channel) gate folds into
    the columns of W2/W3 (per-batch fp8 weight copies), leaving every
    non-matmul pass a plain relu with a scalar immediate scale.
  * All three convs are fp8e4m3 DoubleRow matmuls (K=256 per instruction).
  * x in / y out ship as fp8 (half of bf16 traffic); exact fp32 residual
    `+ x` applied on the host.
  * DMA descriptors must be >=512B or the DMA bus pays a 2x penalty: compute
    chunks are 512 cols (psum tile = exactly one 2KB bank), x loads come in
    1024-col blocks, stores go out in 1024-col groups (on the SP ring).
  * conv3 psum pairs land in adjacent banks ([128,2,512] tiles) so one
    DVE/ACT op drains two m-tiles; drains are split across ACT and DVE.

Sharding: data-parallel over batch B=16 across 8 NeuronCores (2 per core).
"""
import os
import time

import numpy as np

import concourse.bass as bass
import concourse.tile as tile
from concourse import mybir, bass2jax
from concourse.bass2jax import _bass_exec_p, install_neuronx_cc_hook
from contextlib import ExitStack

import jax
from jax.sharding import Mesh, PartitionSpec
from jax.experimental.shard_map import shard_map

B, C, MID, HW = 16, 1024, 256, 56 * 56
NCORES = 8
BPC = B // NCORES            # batches per core
KO = C // 128                # 8 input k-tiles
J1 = KO // 2                 # 4 DoubleRow k-pairs for conv1
M2 = MID // 128              # 2 mid m-tiles
f32 = mybir.dt.float32
fp8 = mybir.dt.float8e4

CHUNK_W = 512                # compute chunk (psum bank = 512 fp32)
BLOCK_W = 1024               # x load / y store granularity (512B+ DMA lines)
# per-batch chunk offsets/widths: 6x512 + 64
CHUNK_OFFS = [(o, min(CHUNK_W, HW - o)) for o in range(0, HW, CHUNK_W)]
BLOCK_OFFS = [(o, min(BLOCK_W, HW - o)) for o in range(0, HW, BLOCK_W)]
NCH = len(CHUNK_OFFS)        # 7 per batch
NBL = len(BLOCK_OFFS)        # 4 per batch

# Power-of-two scales: value*S stored in e4m3. Folded into the matmul
# operands on the host and undone by the relu-pass scalar immediates.
SX, SW1, SR1 = 16.0, 256.0, 16.0
SW2, SR2 = 256.0, 32.0
SW3, SY = 256.0, 32.0
S1 = SR1 / (SX * SW1)        # 1/256
S2 = SR2 / (SR1 * SW2)       # 1/128
S3 = SY / (SR2 * SW3)        # 1/256

# engine per drain op: 4 r-slots (r1m0,r1m1,r2m0,r2m1) + 4 conv3 pair-slots
R_PLAN = os.environ.get("BOTTLENECK_RPLAN", "aaaa")
Y_PLAN = os.environ.get("BOTTLENECK_YPLAN", "avvv")

_EVS_CAP = 2


def _split_excess_waits(nc):
    """This container's walrus accepts only 1 sync-wait slot on most ISA
    structs while Tile emits 2-3; hoist the excess onto preceding
    InstEventSemaphore ops on the same (FIFO) engine queue."""
    counter = [0]
    for fn in nc.m.functions:
        for blk in fn.blocks:
            new_insts = []
            for inst in blk.instructions:
                si = inst.sync_info
                waits = list(si.on_wait) if si is not None else []
                cap = _EVS_CAP if type(inst).__name__ == "InstEventSemaphore" else 1
                if len(waits) > cap:
                    excess, keep = waits[: len(waits) - cap], waits[len(waits) - cap:]
                    while excess:
                        chunk, excess = excess[:_EVS_CAP], excess[_EVS_CAP:]
                        counter[0] += 1
                        new_insts.append(mybir.InstEventSemaphore(
                            name=f"EVSW-{counter[0]}-{inst.name}",
                            engine=inst.engine,
                            ins=[], outs=[],
                            sync_info=mybir.SyncInfo(on_wait=list(chunk), on_update=[]),
                        ))
                    inst.sync_info = mybir.SyncInfo(
                        on_wait=keep, on_update=list(si.on_update))
                new_insts.append(inst)
            blk.instructions = new_insts


def build_bass(repeat: int = 1) -> bass.Bass:
    nc = bass.Bass(trn_type="TRN2")
    xs = nc.dram_tensor("xs", [BPC, C, HW], fp8, kind="ExternalInput")
    w1t = nc.dram_tensor("w1t", [J1, 2, M2, 128, 128], fp8, kind="ExternalInput")
    w2g = nc.dram_tensor("w2g", [BPC, 2, M2, 128, 128], fp8, kind="ExternalInput")
    w3g = nc.dram_tensor("w3g", [BPC, 2, KO, 128, 128], fp8, kind="ExternalInput")
    # Device returns relu(W3g r2)*SY in fp8; host applies + x in fp32.
    ys = nc.dram_tensor("ys", [BPC, C, HW], fp8, kind="ExternalOutput")

    Relu = mybir.ActivationFunctionType.Relu
    DR = mybir.MatmulPerfMode.DoubleRow

    # flat chunk list: (batch, col_off, width, block_idx, group_idx)
    chunks = []
    for b in range(BPC):
        for ci, (off, w) in enumerate(CHUNK_OFFS):
            chunks.append((b, off, w, b * NBL + off // BLOCK_W))
    n = len(chunks)
    # store groups: chunks sharing (batch, block); store fires on last chunk
    grp_of = [c[3] for c in chunks]

    with tile.TileContext(nc) as tc, ExitStack() as ctx:
        wpool = ctx.enter_context(tc.tile_pool(name="w", bufs=1))
        xpool = ctx.enter_context(tc.tile_pool(name="x", bufs=3))
        rpool = ctx.enter_context(tc.tile_pool(name="r", bufs=4))
        opool = ctx.enter_context(tc.tile_pool(name="o", bufs=2))
        # 8 PSUM banks: conv1+conv2 share a single-bank pool (4 allocs/chunk,
        # reuse distance = 1 full chunk); conv3 uses 2-bank pair tiles x2.
        pp12 = ctx.enter_context(tc.tile_pool(name="pp12", bufs=4, space="PSUM"))
        pp3 = ctx.enter_context(tc.tile_pool(name="pp3", bufs=2, space="PSUM"))

        # weights ride the ACT ring (startup only); x loads + y stores ride SP.
        w1_sb = wpool.tile([128, J1, 2, M2, 128], fp8, tag="w1")
        nc.scalar.dma_start(w1_sb[:], w1t[:].rearrange("j i m p c -> p j i m c"))
        w2_sb = wpool.tile([128, BPC, 2, M2, 128], fp8, tag="w2")
        nc.scalar.dma_start(w2_sb[:], w2g[:].rearrange("b i m p c -> p b i m c"))
        w3_sb = wpool.tile([128, BPC, 2, KO, 128], fp8, tag="w3")
        nc.scalar.dma_start(w3_sb[:], w3g[:].rearrange("b i m p c -> p b i m c"))

        def emit_load(bi, halves=1):
            b, (boff, bw) = bi // NBL, BLOCK_OFFS[bi % NBL]
            x_t = xpool.tile([128, KO, BLOCK_W], fp8, tag="xt")
            src = xs[b][:, boff:boff + bw].rearrange("(ko p) n -> p ko n", p=128)
            step = KO // halves
            for h in range(halves):
                nc.sync.dma_start(x_t[:, h * step:(h + 1) * step, :bw],
                                  src[:, h * step:(h + 1) * step, :])
            return x_t

        def emit_conv1(i, x_t):
            b, off, w, bi = chunks[i]
            o = off - BLOCK_OFFS[bi % NBL][0]
            ps1 = []
            for m in range(M2):
                ps = pp12.tile([128, CHUNK_W], f32, tag="ps12")
                for j in range(J1):
                    nc.tensor.matmul(ps[:, :w], w1_sb[:, j, :, m, :],
                                     x_t[:, 2 * j:2 * j + 2, o:o + w],
                                     start=(j == 0), stop=(j == J1 - 1),
                                     perf_mode=DR)
                ps1.append(ps)
            return ps1

        def _drain(which, dst, src, scale):
            if which == "v":
                nc.vector.tensor_scalar(dst, src, scale, 0.0,
                                        mybir.AluOpType.mult,
                                        mybir.AluOpType.max)
            else:
                nc.scalar.activation(dst, src, Relu, scale=scale)

        def emit_fin_a(i, ps1):
            """r1 = relu(S1*ps1) fp8; conv2 (DoubleRow); r2 = relu(S2*ps2)."""
            b, off, w, bi = chunks[i]
            r1 = rpool.tile([128, 2, CHUNK_W], fp8, tag="r1")
            for m in range(M2):
                _drain(R_PLAN[m], r1[:, m, :w], ps1[m][:, :w], S1)
            r2 = rpool.tile([128, 2, CHUNK_W], fp8, tag="r2")
            for m in range(M2):
                ps = pp12.tile([128, CHUNK_W], f32, tag="ps12")
                nc.tensor.matmul(ps[:, :w], w2_sb[:, b, :, m, :],
                                 r1[:, :, :w], start=True, stop=True,
                                 perf_mode=DR)
                _drain(R_PLAN[2 + m], r2[:, m, :w], ps[:, :w], S2)
            return r2

        def emit_fin_b(i, r2, o_t, last):
            """conv3 (DoubleRow) into 2-bank psum pairs; fused relu drains;
            store the o_t group when its last chunk completes."""
            b, off, w, bi = chunks[i]
            boff, bw = BLOCK_OFFS[bi % NBL]
            o = off - boff
            for pr in range(KO // 2):
                ps = pp3.tile([128, 2, CHUNK_W], f32, tag="ps3")
                for m in range(2):
                    nc.tensor.matmul(ps[:, m, :w], w3_sb[:, b, :, 2 * pr + m, :],
                                     r2[:, :, :w], start=True, stop=True,
                                     perf_mode=DR)
                _drain(Y_PLAN[pr], o_t[:, 2 * pr:2 * pr + 2, o:o + w],
                       ps[:, :, :w], S3)
            if last and (i + 1 == n or grp_of[i + 1] != grp_of[i]):
                dst = ys[b][:, boff:boff + bw].rearrange("(m p) n -> p m n", p=128)
                nc.sync.dma_start(dst, o_t[:, :, :bw])

        for r in range(repeat):
            last = r == repeat - 1
            xts = {}            # block_idx -> x tile
            ots = {}            # group idx -> o tile
            ps1s = {}
            r2s = {}
            for bi in range(min(2, NBL * BPC)):
                xts[bi] = emit_load(bi, halves=2 if bi == 0 else 1)
            ps1s[0] = emit_conv1(0, xts[chunks[0][3]])
            r2s[0] = emit_fin_a(0, ps1s.pop(0))
            if n > 1:
                ps1s[1] = emit_conv1(1, xts[chunks[1][3]])
            for i in range(n):
                # prefetch the block for chunk i+4 (~2 blocks ahead of use)
                if i + 4 < n:
                    nbi = chunks[i + 4][3]
                    if nbi not in xts:
                        xts[nbi] = emit_load(nbi)
                if i + 1 < n:
                    r2s[i + 1] = emit_fin_a(i + 1, ps1s.pop(i + 1))
                if i + 2 < n:
                    ps1s[i + 2] = emit_conv1(i + 2, xts[chunks[i + 2][3]])
                gi = grp_of[i]
                if gi not in ots:
                    o_t = opool.tile([128, KO, BLOCK_W], fp8, tag="ot")
                    ots = {gi: o_t}
                emit_fin_b(i, r2s.pop(i), ots[gi], last)
    return nc


class _Exec:
    """Compile-once PJRT executor for the SPMD bass program (axon backend)."""

    def __init__(self, nc, n_cores):
        install_neuronx_cc_hook()
        self.n_cores = n_cores
        partition_name = nc.partition_id_tensor.name if nc.partition_id_tensor else None
        in_names, out_names, out_avals, zero_outs = [], [], [], []
        for alloc in nc.m.functions[0].allocations:
            if not isinstance(alloc, mybir.MemoryLocationSet):
                continue
            name = alloc.memorylocations[0].name
            if alloc.kind == "ExternalInput":
                if name != partition_name:
                    in_names.append(name)
            elif alloc.kind == "ExternalOutput":
                shape = tuple(alloc.tensor_shape)
                dtype = mybir.dt.np(alloc.dtype)
                out_names.append(name)
                out_avals.append(jax.core.ShapedArray(shape, dtype))
                zero_outs.append(np.zeros(shape, dtype))
        self.in_names, self.out_names, self.zero_outs = in_names, out_names, zero_outs
        n_params = len(in_names)
        all_in = list(in_names) + list(out_names)
        if partition_name is not None:
            all_in.append(partition_name)

        def _body(*args):
            operands = list(args)
            if partition_name is not None:
                operands.append(bass2jax.partition_id_tensor())
            return tuple(_bass_exec_p.bind(
                *operands,
                out_avals=tuple(out_avals),
                in_names=tuple(all_in),
                out_names=tuple(out_names),
                lowering_input_output_aliases=(),
                sim_require_finite=True,
                sim_require_nnan=True,
                nc=nc,
            ))

        devices = jax.devices()[:n_cores]
        assert len(devices) == n_cores, f"need {n_cores} cores, have {len(jax.devices())}"
        mesh = Mesh(np.asarray(devices), ("core",))
        specs = (PartitionSpec("core"),) * (n_params + len(out_names))
        self._fn = jax.jit(
            shard_map(_body, mesh=mesh, in_specs=specs,
                      out_specs=(PartitionSpec("core"),) * len(out_names),
                      check_rep=False),
            keep_unused=True,
        )

    def stage(self, in_maps):
        per_core = [[np.asarray(m[n]) for n in self.in_names] for m in in_maps]
        args = [np.concatenate([per_core[c][i] for c in range(self.n_cores)], axis=0)
                for i in range(len(self.in_names))]
        args += [np.zeros((self.n_cores * z.shape[0], *z.shape[1:]), z.dtype)
                 for z in self.zero_outs]
        return args

    def run_staged(self, args):
        out = self._fn(*args)
        jax.block_until_ready(out)
        return out

    def fetch(self, out_arrs):
        return [
            {n: np.asarray(out_arrs[i]).reshape(self.n_cores, *self.zero_outs[i].shape)[c]
             for i, n in enumerate(self.out_names)}
            for c in range(self.n_cores)
        ]


_EXEC_CACHE = {}


def _get_exec(repeat: int = 1):
    if repeat not in _EXEC_CACHE:
        nc = build_bass(repeat)
        _split_excess_waits(nc)
        _EXEC_CACHE[repeat] = _Exec(nc, NCORES)
    return _EXEC_CACHE[repeat]


def _prepare_in_maps(x, gate_values, W1, b1, W2, b2, W3, b3):
    import ml_dtypes
    e4m3 = ml_dtypes.float8_e4m3
    x = np.asarray(x, dtype=np.float32)
    gate = np.asarray(gate_values, dtype=np.float32)
    W1 = np.asarray(W1, dtype=np.float32)
    W2 = np.asarray(W2, dtype=np.float32)
    W3 = np.asarray(W3, dtype=np.float32)
    # Biases are structurally zero in this problem; the device program
    # assumes so (pure relu passes with immediate scales).
    for bv in (b1, b2, b3):
        assert not np.any(np.asarray(bv)), "nonzero bias unsupported"

    xs_all = (x.reshape(B, C, HW) * SX).astype(e4m3)
    g_all = np.maximum(gate, 0.0)                      # [B, MID]

    # lhsT DoubleRow tiles: [j, i, m, p, c] = Wq.T[(2j+i)*128+p, m*128+c]
    w1t = np.ascontiguousarray(
        (W1 * SW1).astype(e4m3).T.reshape(J1, 2, 128, M2, 128)
        .transpose(0, 1, 3, 2, 4))

    in_maps = []
    for c in range(NCORES):
        w2l, w3l = [], []
        for bl in range(BPC):
            g = g_all[c * BPC + bl]
            w2q = (W2 * g[None, :] * SW2).astype(e4m3)   # [MID, MID]
            w3q = (W3 * g[None, :] * SW3).astype(e4m3)   # [C, MID]
            w2l.append(w2q.T.reshape(2, 128, M2, 128).transpose(0, 2, 1, 3))
            w3l.append(w3q.T.reshape(2, 128, KO, 128).transpose(0, 2, 1, 3))
        in_maps.append({
            "xs": xs_all[c * BPC:(c + 1) * BPC],
            "w1t": w1t,
            "w2g": np.ascontiguousarray(np.stack(w2l)),
            "w3g": np.ascontiguousarray(np.stack(w3l)),
        })
    return in_maps


def kernel(x, gate_values, W1, b1, W2, b2, W3, b3):
    in_maps = _prepare_in_maps(x, gate_values, W1, b1, W2, b2, W3, b3)
    ex = _get_exec(int(os.environ.get("BOTTLENECK_REPEAT", "1")))
    args = ex.stage(in_maps)
    try:
        out_arrs = ex.run_staged(args)
    except Exception:
        time.sleep(2.0)  # transient device wedge: retry once
        out_arrs = ex.run_staged(args)
    outs = ex.fetch(out_arrs)
    relu3 = np.concatenate([o["ys"] for o in outs], axis=0).astype(np.float32)
    y = np.asarray(x, dtype=np.float32).reshape(B, C, HW) + relu3 * (1.0 / SY)
    return y.reshape(B, C, 56, 56)


# revision 6
# speedup vs baseline: 1.4593x; 1.3524x over previous
"""TRN2 Bass kernel for nn_BottleneckA — fp8 DoubleRow, DMA-efficient version.

Computation (reference):
    h1 = relu(g * (W1 @ x))    g = relu(gate)   per (batch, mid-channel)
    h2 = relu(g * (W2 @ h1))
    y  = relu(W3 @ h2) + x     (all biases are zero in this problem)

Design (evolved from the bf16/f32r baseline via sim-trace analysis):
  * relu(g*z) = g*relu(z) for g>=0: the per-(batch,channel) gate folds into
    the columns of W2/W3 (per-batch fp8 weight copies), leaving every
    non-matmul pass a plain relu with a scalar immediate scale.
  * All three convs are fp8e4m3 DoubleRow matmuls (K=256 per instruction).
  * x in / y out ship as fp8 (half of bf16 traffic); exact fp32 residual
    `+ x` applied on the host.
  * DMA descriptors must be >=512B or the DMA bus pays a 2x penalty: compute
    chunks are 512 cols (psum tile = exactly one 2KB bank), x loads come in
    1024-col blocks, stores go out in 1024-col groups (on the SP ring).
  * conv3 psum pairs land in adjacent banks ([128,2,512] tiles) so one
    DVE/ACT op drains two m-tiles; drains are split across ACT and DVE.

Sharding: data-parallel over batch B=16 across 8 NeuronCores (2 per core).
"""
import os
import time

import numpy as np

import concourse.bass as bass
import concourse.tile as tile
from concourse import mybir, bass2jax
from concourse.bass2jax import _bass_exec_p, install_neuronx_cc_hook
from contextlib import ExitStack

import jax
from jax.sharding import Mesh, PartitionSpec
from jax.experimental.shard_map import shard_map

B, C, MID, HW = 16, 1024, 256, 56 * 56
NCORES = 8
BPC = B // NCORES            # batches per core
KO = C // 128                # 8 input k-tiles
J1 = KO // 2                 # 4 DoubleRow k-pairs for conv1
M2 = MID // 128              # 2 mid m-tiles
f32 = mybir.dt.float32
fp8 = mybir.dt.float8e4

CHUNK_W = 512                # compute chunk (psum bank = 512 fp32)
BLOCK_W = 1024               # x load / y store granularity (512B+ DMA lines)
# per-batch chunk offsets/widths: 6x512 + 64
CHUNK_OFFS = [(o, min(CHUNK_W, HW - o)) for o in range(0, HW, CHUNK_W)]
BLOCK_OFFS = [(o, min(BLOCK_W, HW - o)) for o in range(0, HW, BLOCK_W)]
NCH = len(CHUNK_OFFS)        # 7 per batch
NBL = len(BLOCK_OFFS)        # 4 per batch

# Power-of-two scales: value*S stored in e4m3. Folded into the matmul
# operands on the host and undone by the relu-pass scalar immediates.
SX, SW1, SR1 = 16.0, 256.0, 16.0
SW2, SR2 = 256.0, 32.0
SW3, SY = 256.0, 32.0
S1 = SR1 / (SX * SW1)        # 1/256
S2 = SR2 / (SR1 * SW2)       # 1/128
S3 = SY / (SR2 * SW3)        # 1/256

# engine per drain op: 4 r-slots (r1m0,r1m1,r2m0,r2m1) + 4 conv3 pair-slots
R_PLAN = os.environ.get("BOTTLENECK_RPLAN", "aaaa")
Y_PLAN = os.environ.get("BOTTLENECK_YPLAN", "avvv")
# probe mode: emit conv3 matmuls but skip drains+stores (PE/ACT pipeline timing)
PROBE = bool(int(os.environ.get("BOTTLENECK_PROBE", "0")))

_EVS_CAP = 2


def _split_excess_waits(nc):
    """This container's walrus accepts only 1 sync-wait slot on most ISA
    structs while Tile emits 2-3; hoist the excess onto preceding
    InstEventSemaphore ops on the same (FIFO) engine queue."""
    counter = [0]
    for fn in nc.m.functions:
        for blk in fn.blocks:
            new_insts = []
            for inst in blk.instructions:
                si = inst.sync_info
                waits = list(si.on_wait) if si is not None else []
                cap = _EVS_CAP if type(inst).__name__ == "InstEventSemaphore" else 1
                if len(waits) > cap:
                    excess, keep = waits[: len(waits) - cap], waits[len(waits) - cap:]
                    while excess:
                        chunk, excess = excess[:_EVS_CAP], excess[_EVS_CAP:]
                        counter[0] += 1
                        new_insts.append(mybir.InstEventSemaphore(
                            name=f"EVSW-{counter[0]}-{inst.name}",
                            engine=inst.engine,
                            ins=[], outs=[],
                            sync_info=mybir.SyncInfo(on_wait=list(chunk), on_update=[]),
                        ))
                    inst.sync_info = mybir.SyncInfo(
                        on_wait=keep, on_update=list(si.on_update))
                new_insts.append(inst)
            blk.instructions = new_insts


def build_bass(repeat: int = 1) -> bass.Bass:
    nc = bass.Bass(trn_type="TRN2")
    xs = nc.dram_tensor("xs", [BPC, C, HW], fp8, kind="ExternalInput")
    w1t = nc.dram_tensor("w1t", [J1, 2, M2, 128, 128], fp8, kind="ExternalInput")
    w2g = nc.dram_tensor("w2g", [BPC, 2, M2, 128, 128], fp8, kind="ExternalInput")
    w3g = nc.dram_tensor("w3g", [BPC, 2, KO, 128, 128], fp8, kind="ExternalInput")
    # Device returns relu(W3g r2)*SY in fp8; host applies + x in fp32.
    ys = nc.dram_tensor("ys", [BPC, C, HW], fp8, kind="ExternalOutput")

    Relu = mybir.ActivationFunctionType.Relu
    DR = mybir.MatmulPerfMode.DoubleRow

    # flat chunk list: (batch, col_off, width, block_idx, group_idx)
    chunks = []
    for b in range(BPC):
        for ci, (off, w) in enumerate(CHUNK_OFFS):
            chunks.append((b, off, w, b * NBL + off // BLOCK_W))
    n = len(chunks)
    # store groups: chunks sharing (batch, block); store fires on last chunk
    grp_of = [c[3] for c in chunks]

    with tile.TileContext(nc) as tc, ExitStack() as ctx:
        wpool = ctx.enter_context(tc.tile_pool(name="w", bufs=1))
        xpool = ctx.enter_context(tc.tile_pool(name="x", bufs=3))
        rpool = ctx.enter_context(tc.tile_pool(name="r", bufs=4))
        opool = ctx.enter_context(tc.tile_pool(name="o", bufs=2))
        # 8 PSUM banks: conv1+conv2 share a single-bank pool (4 allocs/chunk,
        # reuse distance = 1 full chunk); conv3 uses 2-bank pair tiles x2.
        pp12 = ctx.enter_context(tc.tile_pool(name="pp12", bufs=4, space="PSUM"))
        pp3 = ctx.enter_context(tc.tile_pool(name="pp3", bufs=2, space="PSUM"))

        # weights ride the ACT ring (startup only); x loads + y stores ride SP.
        w1_sb = wpool.tile([128, J1, 2, M2, 128], fp8, tag="w1")
        nc.scalar.dma_start(w1_sb[:], w1t[:].rearrange("j i m p c -> p j i m c"))
        w2_sb = wpool.tile([128, BPC, 2, M2, 128], fp8, tag="w2")
        nc.scalar.dma_start(w2_sb[:], w2g[:].rearrange("b i m p c -> p b i m c"))
        w3_sb = wpool.tile([128, BPC, 2, KO, 128], fp8, tag="w3")
        nc.scalar.dma_start(w3_sb[:], w3g[:].rearrange("b i m p c -> p b i m c"))

        def emit_load(bi, halves=1):
            b, (boff, bw) = bi // NBL, BLOCK_OFFS[bi % NBL]
            x_t = xpool.tile([128, KO, BLOCK_W], fp8, tag="xt")
            src = xs[b][:, boff:boff + bw].rearrange("(ko p) n -> p ko n", p=128)
            step = KO // halves
            for h in range(halves):
                nc.sync.dma_start(x_t[:, h * step:(h + 1) * step, :bw],
                                  src[:, h * step:(h + 1) * step, :])
            return x_t

        def emit_conv1(i, x_t):
            b, off, w, bi = chunks[i]
            o = off - BLOCK_OFFS[bi % NBL][0]
            ps1 = []
            for m in range(M2):
                ps = pp12.tile([128, CHUNK_W], f32, tag="ps12")
                for j in range(J1):
                    nc.tensor.matmul(ps[:, :w], w1_sb[:, j, :, m, :],
                                     x_t[:, 2 * j:2 * j + 2, o:o + w],
                                     start=(j == 0), stop=(j == J1 - 1),
                                     perf_mode=DR)
                ps1.append(ps)
            return ps1

        def _drain(which, dst, src, scale):
            if which == "v":
                nc.vector.tensor_scalar(dst, src, scale, 0.0,
                                        mybir.AluOpType.mult,
                                        mybir.AluOpType.max)
            else:
                nc.scalar.activation(dst, src, Relu, scale=scale)

        def emit_fin_a(i, ps1):
            """r1 = relu(S1*ps1) fp8; conv2 (DoubleRow); r2 = relu(S2*ps2)."""
            b, off, w, bi = chunks[i]
            r1 = rpool.tile([128, 2, CHUNK_W], fp8, tag="r1")
            for m in range(M2):
                _drain(R_PLAN[m], r1[:, m, :w], ps1[m][:, :w], S1)
            r2 = rpool.tile([128, 2, CHUNK_W], fp8, tag="r2")
            for m in range(M2):
                ps = pp12.tile([128, CHUNK_W], f32, tag="ps12")
                nc.tensor.matmul(ps[:, :w], w2_sb[:, b, :, m, :],
                                 r1[:, :, :w], start=True, stop=True,
                                 perf_mode=DR)
                _drain(R_PLAN[2 + m], r2[:, m, :w], ps[:, :w], S2)
            return r2

        def emit_fin_b(i, r2, o_t, last):
            """conv3 (DoubleRow) into 2-bank psum pairs; fused relu drains;
            store the o_t group when its last chunk completes."""
            b, off, w, bi = chunks[i]
            boff, bw = BLOCK_OFFS[bi % NBL]
            o = off - boff
            for pr in range(KO // 2):
                ps = pp3.tile([128, 2, CHUNK_W], f32, tag="ps3")
                for m in range(2):
                    nc.tensor.matmul(ps[:, m, :w], w3_sb[:, b, :, 2 * pr + m, :],
                                     r2[:, :, :w], start=True, stop=True,
                                     perf_mode=DR)
                if not PROBE:
                    _drain(Y_PLAN[pr], o_t[:, 2 * pr:2 * pr + 2, o:o + w],
                           ps[:, :, :w], S3)
            if last and not PROBE and (i + 1 == n or grp_of[i + 1] != grp_of[i]):
                dst = ys[b][:, boff:boff + bw].rearrange("(m p) n -> p m n", p=128)
                nc.sync.dma_start(dst, o_t[:, :, :bw])

        for r in range(repeat):
            last = r == repeat - 1
            xts = {}            # block_idx -> x tile
            ots = {}            # group idx -> o tile
            ps1s = {}
            r2s = {}
            for bi in range(min(2, NBL * BPC)):
                xts[bi] = emit_load(bi, halves=2 if bi == 0 else 1)
            ps1s[0] = emit_conv1(0, xts[chunks[0][3]])
            r2s[0] = emit_fin_a(0, ps1s.pop(0))
            if n > 1:
                ps1s[1] = emit_conv1(1, xts[chunks[1][3]])
            for i in range(n):
                # prefetch the block for chunk i+4 (~2 blocks ahead of use)
                if i + 4 < n:
                    nbi = chunks[i + 4][3]
                    if nbi not in xts:
                        xts[nbi] = emit_load(nbi)
                if i + 1 < n:
                    r2s[i + 1] = emit_fin_a(i + 1, ps1s.pop(i + 1))
                if i + 2 < n:
                    ps1s[i + 2] = emit_conv1(i + 2, xts[chunks[i + 2][3]])
                gi = grp_of[i]
                if gi not in ots:
                    o_t = opool.tile([128, KO, BLOCK_W], fp8, tag="ot")
                    ots = {gi: o_t}
                emit_fin_b(i, r2s.pop(i), ots[gi], last)
    return nc


class _Exec:
    """Compile-once PJRT executor for the SPMD bass program (axon backend)."""

    def __init__(self, nc, n_cores):
        install_neuronx_cc_hook()
        self.n_cores = n_cores
        partition_name = nc.partition_id_tensor.name if nc.partition_id_tensor else None
        in_names, out_names, out_avals, zero_outs = [], [], [], []
        for alloc in nc.m.functions[0].allocations:
            if not isinstance(alloc, mybir.MemoryLocationSet):
                continue
            name = alloc.memorylocations[0].name
            if alloc.kind == "ExternalInput":
                if name != partition_name:
                    in_names.append(name)
            elif alloc.kind == "ExternalOutput":
                shape = tuple(alloc.tensor_shape)
                dtype = mybir.dt.np(alloc.dtype)
                out_names.append(name)
                out_avals.append(jax.core.ShapedArray(shape, dtype))
                zero_outs.append(np.zeros(shape, dtype))
        self.in_names, self.out_names, self.zero_outs = in_names, out_names, zero_outs
        n_params = len(in_names)
        all_in = list(in_names) + list(out_names)
        if partition_name is not None:
            all_in.append(partition_name)

        def _body(*args):
            operands = list(args)
            if partition_name is not None:
                operands.append(bass2jax.partition_id_tensor())
            return tuple(_bass_exec_p.bind(
                *operands,
                out_avals=tuple(out_avals),
                in_names=tuple(all_in),
                out_names=tuple(out_names),
                lowering_input_output_aliases=(),
                sim_require_finite=True,
                sim_require_nnan=True,
                nc=nc,
            ))

        devices = jax.devices()[:n_cores]
        assert len(devices) == n_cores, f"need {n_cores} cores, have {len(jax.devices())}"
        mesh = Mesh(np.asarray(devices), ("core",))
        specs = (PartitionSpec("core"),) * (n_params + len(out_names))
        self._fn = jax.jit(
            shard_map(_body, mesh=mesh, in_specs=specs,
                      out_specs=(PartitionSpec("core"),) * len(out_names),
                      check_rep=False),
            keep_unused=True,
        )

    def stage(self, in_maps):
        per_core = [[np.asarray(m[n]) for n in self.in_names] for m in in_maps]
        args = [np.concatenate([per_core[c][i] for c in range(self.n_cores)], axis=0)
                for i in range(len(self.in_names))]
        args += [np.zeros((self.n_cores * z.shape[0], *z.shape[1:]), z.dtype)
                 for z in self.zero_outs]
        return args

    def run_staged(self, args):
        out = self._fn(*args)
        jax.block_until_ready(out)
        return out

    def fetch(self, out_arrs):
        return [
            {n: np.asarray(out_arrs[i]).reshape(self.n_cores, *self.zero_outs[i].shape)[c]
             for i, n in enumerate(self.out_names)}
            for c in range(self.n_cores)
        ]


_EXEC_CACHE = {}


def _get_exec(repeat: int = 1):
    if repeat not in _EXEC_CACHE:
        nc = build_bass(repeat)
        _split_excess_waits(nc)
        _EXEC_CACHE[repeat] = _Exec(nc, NCORES)
    return _EXEC_CACHE[repeat]


def _prepare_in_maps(x, gate_values, W1, b1, W2, b2, W3, b3):
    import ml_dtypes
    e4m3 = ml_dtypes.float8_e4m3
    x = np.asarray(x, dtype=np.float32)
    gate = np.asarray(gate_values, dtype=np.float32)
    W1 = np.asarray(W1, dtype=np.float32)
    W2 = np.asarray(W2, dtype=np.float32)
    W3 = np.asarray(W3, dtype=np.float32)
    # Biases are structurally zero in this problem; the device program
    # assumes so (pure relu passes with immediate scales).
    for bv in (b1, b2, b3):
        assert not np.any(np.asarray(bv)), "nonzero bias unsupported"

    xs_all = (x.reshape(B, C, HW) * SX).astype(e4m3)
    g_all = np.maximum(gate, 0.0)                      # [B, MID]

    # lhsT DoubleRow tiles: [j, i, m, p, c] = Wq.T[(2j+i)*128+p, m*128+c]
    w1t = np.ascontiguousarray(
        (W1 * SW1).astype(e4m3).T.reshape(J1, 2, 128, M2, 128)
        .transpose(0, 1, 3, 2, 4))

    in_maps = []
    for c in range(NCORES):
        w2l, w3l = [], []
        for bl in range(BPC):
            g = g_all[c * BPC + bl]
            w2q = (W2 * g[None, :] * SW2).astype(e4m3)   # [MID, MID]
            w3q = (W3 * g[None, :] * SW3).astype(e4m3)   # [C, MID]
            w2l.append(w2q.T.reshape(2, 128, M2, 128).transpose(0, 2, 1, 3))
            w3l.append(w3q.T.reshape(2, 128, KO, 128).transpose(0, 2, 1, 3))
        in_maps.append({
            "xs": xs_all[c * BPC:(c + 1) * BPC],
            "w1t": w1t,
            "w2g": np.ascontiguousarray(np.stack(w2l)),
            "w3g": np.ascontiguousarray(np.stack(w3l)),
        })
    return in_maps


def kernel(x, gate_values, W1, b1, W2, b2, W3, b3):
    in_maps = _prepare_in_maps(x, gate_values, W1, b1, W2, b2, W3, b3)
    ex = _get_exec(int(os.environ.get("BOTTLENECK_REPEAT", "1")))
    args = ex.stage(in_maps)
    try:
        out_arrs = ex.run_staged(args)
    except Exception:
        time.sleep(2.0)  # transient device wedge: retry once
        out_arrs = ex.run_staged(args)
    outs = ex.fetch(out_arrs)
    relu3 = np.concatenate([o["ys"] for o in outs], axis=0).astype(np.float32)
    y = np.asarray(x, dtype=np.float32).reshape(B, C, HW) + relu3 * (1.0 / SY)
    return y.reshape(B, C, 56, 56)
